# revision 4
# baseline (speedup 1.0000x reference)
"""Trainium2 Bass kernel for nn_LocalConnectivity (diamond stencil, B=64, H=W=1024).

out[b,h,w] = sum over offsets (dx,dy), 1 <= |dx|+|dy| <= 5, of
             exp(-(|dx|+|dy|)) * x[b, (h-dx) % H, (w-dy) % W]

Strategy (per core, 8 images each, batch-sharded over 8 NeuronCores):
  Group the 60 taps by horizontal offset dy in [-5, 5]. For each dy the
  vertical tap profile is a banded matrix G_{|dy|} (half-width 5-|dy|).
  The host pre-pads each image circularly by 5 in both axes to
  (1034, 1034), so every h-window is a single contiguous DMA and the
  w-halo is already materialized. Per 118-row output window, accumulate
  11 TensorEngine matmuls per 512-col block into PSUM:
      psum[:M, :] += vband_j[:K, :M].T @ tile[:K, 5+off-dy : +512]
  float32r matmuls run at 1 col/cycle (vs 4 for fp32). PSUM is evacuated
  by ScalarE/VectorE copies and DMA'd back to HBM.
"""

import math

import numpy as np

B_TOTAL = 64
B_PER_CORE = 8
N_CORES = 8
H = 1024
W = 1024
PAD = 5
HP = H + 2 * PAD  # 1034
WP = W + 2 * PAD  # 1034
MW = 118  # output rows per h-window (last window gets the remainder: 80)
NBLK = 512  # w-block streamed per matmul (PSUM bank = 512 fp32)
DYS = [0, -1, 1, -2, 2, -3, 3, -4, 4, -5, 5]

_CACHE = {}


def _windows():
    ws = []
    h0 = 0
    while h0 < H:
        m = min(MW, H - h0)
        ws.append((h0, m))
        h0 += m
    return ws


def _build_vbands() -> np.ndarray:
    """vb[k, j*MW + m] = G_j(dx) at dx = m + 5 - k.

    G_j(dx) = exp(-(|dx|+j)) for |dx| <= 5-j, excluding the (j=0, dx=0) tap.
    lhsT layout: [K=128 partitions, 6*MW free], band j in cols [j*MW, j*MW+MW).
    """
    vb = np.zeros((128, 6 * MW), np.float32)
    for j in range(6):
        for m in range(MW):
            for dx in range(-(5 - j), 5 - j + 1):
                if j == 0 and dx == 0:
                    continue
                k = m + 5 - dx
                vb[k, j * MW + m] = math.exp(-(abs(dx) + j))
    return vb


def _build_program():
    from concourse.bacc import Bacc
    import concourse.mybir as mybir
    from concourse.tile import TileContext

    f32 = mybir.dt.float32
    f32r = mybir.dt.float32r

    nc = Bacc("TRN2", target_bir_lowering=False, debug=False)
    x = nc.dram_tensor("x", [B_PER_CORE, HP, WP], f32r, kind="ExternalInput")
    vb = nc.dram_tensor("vb", [128, 6 * MW], f32r, kind="ExternalInput")
    y = nc.dram_tensor("y", [B_PER_CORE, H, W], f32, kind="ExternalOutput")

    with TileContext(nc) as tc:
        with (
            tc.tile_pool(name="bands", bufs=1) as bpool,
            tc.tile_pool(name="inp", bufs=3) as ipool,
            tc.tile_pool(name="outp", bufs=3) as opool,
            tc.tile_pool(name="ps", bufs=4, space="PSUM") as pspool,
        ):
            vbt = bpool.tile([128, 6 * MW], f32r, name="vbt")
            nc.sync.dma_start(out=vbt[:, :], in_=vb[:, :])

            for b in range(B_PER_CORE):
                for h0, Mw in _windows():
                    Kw = Mw + 10
                    it = ipool.tile([128, WP], f32r, tag="it", name="it")
                    # padded rows h0 .. h0+Kw = image rows h0-5 .. h0+Mw+5
                    nc.sync.dma_start(out=it[:Kw, :], in_=x[b, h0 : h0 + Kw, :])

                    ot = opool.tile([128, W], f32, tag="ot", name="ot")
                    for wb in range(W // NBLK):
                        ps = pspool.tile([128, NBLK], f32, tag="ps", name="ps")
                        for idx, dy in enumerate(DYS):
                            j = abs(dy)
                            c0 = PAD + NBLK * wb - dy
                            nc.tensor.matmul(
                                ps[:Mw, :],
                                lhsT=vbt[:Kw, j * MW : j * MW + Mw],
                                rhs=it[:Kw, c0 : c0 + NBLK],
                                start=(idx == 0),
                                stop=(idx == len(DYS) - 1),
                            )
                        # evacuate PSUM -> SBUF, alternating engines
                        if wb % 2 == 0:
                            nc.scalar.copy(ot[:Mw, wb * NBLK : (wb + 1) * NBLK], ps[:Mw, :])
                        else:
                            nc.vector.tensor_copy(
                                ot[:Mw, wb * NBLK : (wb + 1) * NBLK], ps[:Mw, :]
                            )
                    nc.sync.dma_start(out=y[b, h0 : h0 + Mw, :], in_=ot[:Mw, :])
    nc.compile()
    return nc


def _get_program():
    if "nc" not in _CACHE:
        _CACHE["nc"] = _build_program()
        _CACHE["vb"] = _build_vbands()
    return _CACHE["nc"], _CACHE["vb"]


def _run(grid_spikes: np.ndarray, **spmd_kwargs):
    """Run the SPMD kernel on the full (64, 1024, 1024) input.

    Returns (output, BassKernelResults)."""
    from concourse.bass_utils import run_bass_kernel_spmd

    nc, vb = _get_program()
    gs = np.ascontiguousarray(grid_spikes, dtype=np.float32)
    assert gs.shape == (B_TOTAL, H, W), gs.shape
    gp = np.pad(gs, ((0, 0), (PAD, PAD), (PAD, PAD)), mode="wrap")
    in_maps = [
        {"x": gp[c * B_PER_CORE : (c + 1) * B_PER_CORE], "vb": vb}
        for c in range(N_CORES)
    ]
    res = run_bass_kernel_spmd(nc, in_maps, core_ids=list(range(N_CORES)), **spmd_kwargs)
    out = np.concatenate([r["y"] for r in res.results], axis=0)
    return out, res


def kernel(grid_spikes: np.ndarray) -> np.ndarray:
    out, _ = _run(grid_spikes)
    return out


# revision 17
# speedup vs baseline: 1.4463x; 1.4463x over previous
"""Trainium2 Bass kernel for nn_LocalConnectivity (diamond stencil, B=64, H=W=1024).

out[b,h,w] = sum over offsets (dx,dy), 1 <= |dx|+|dy| <= 5, of
             exp(-(|dx|+|dy|)) * x[b, (h-dx) % H, (w-dy) % W]

Strategy (per core, 8 images each, batch-sharded over 8 NeuronCores):
  Group the 60 taps by horizontal offset dy in [-5, 5]. For each dy the
  vertical tap profile is a banded matrix G_{|dy|} (half-width 5-|dy|).
  The host pre-pads each image circularly (5 left/top/right, 7 bottom) to
  (1036, 1034). Each image is processed as 9 uniform 114-row output
  windows; per window and 512-col block, 11 TensorEngine matmuls
  accumulate into PSUM:
      psum[:114, :] += vband_j[:124, :114].T @ tile[:124, 5+off-dy : +512]
  (bf16 streams at 1 col/cycle). PSUM is evacuated by ScalarE/VectorE
  copies into fp16 tiles and DMA'd back to HBM. Rows 1024..1025 of the
  padded output are garbage and dropped on the host.

  DMA is batched per image (one ~2.4 MB input transfer with overlapping-
  window access patterns, one ~2.1 MB output transfer): small per-window
  DMAs run at <140 GB/s while >=1 MB transfers reach ~340 GB/s. Input
  DMAs ride the SP HWDGE ring, output DMAs the ACT ring.
"""

import math

import numpy as np

B_TOTAL = 64
B_PER_CORE = 8
N_CORES = 8
H = 1024
W = 1024
PAD = 5
HPX = H + PAD + 7  # 1036 (5 top, 7 bottom: 2 extra rows for the 9th window)
WP = W + 2 * PAD  # 1034
MW = 114  # output rows per h-window
NW = 9  # uniform windows per image; NW*MW = 1026 >= H (2 garbage rows)
KW = MW + 10  # 124 input rows per window
HOUT = NW * MW  # 1026 padded output rows
NBLK = 512  # w-block streamed per matmul (PSUM bank = 512 fp32)
DYS = [0, -1, 1, -2, 2, -3, 3, -4, 4, -5, 5]

DTYPE = "bfloat16"  # matmul input dtype: "bfloat16" or "float32r"
OUT_DTYPE = "float16"  # HBM output dtype: "float16" or "float32"

_CACHE = {}


def _build_vbands() -> np.ndarray:
    """vb[k, j*MW + m] = G_j(dx) at dx = m + 5 - k.

    G_j(dx) = exp(-(|dx|+j)) for |dx| <= 5-j, excluding the (j=0, dx=0) tap.
    lhsT layout: [K=128 partitions, 6*MW free], band j in cols [j*MW, j*MW+MW).
    """
    vb = np.zeros((128, 6 * MW), np.float32)
    for j in range(6):
        for m in range(MW):
            for dx in range(-(5 - j), 5 - j + 1):
                if j == 0 and dx == 0:
                    continue
                k = m + 5 - dx
                vb[k, j * MW + m] = math.exp(-(abs(dx) + j))
    return vb


def _emit_body(nc, mybir, bass, pools, vbt, x, y, in_dt, out_dt, variant="full", fixed_it=None):
    """Emit the per-core compute: all images; batched per-image DMAs.

    variant: "full" | "nodma" (matmuls only, fixed input tile) | "nomm" (DMA only)
    """
    f32 = mybir.dt.float32
    ipool, opool, pspool = pools

    for b in range(B_PER_CORE):
        if variant == "nodma":
            it_all = fixed_it
        else:
            # one batched input DMA: 9 overlapping 124-row windows
            it_all = ipool.tile([128, NW * WP], in_dt, tag="it_all", name="it_all")
            src = bass.AP(
                tensor=x,
                offset=b * HPX * WP,
                ap=[[WP, KW], [MW * WP, NW], [1, WP]],
            )
            dst = it_all.rearrange("p (w c) -> p w c", c=WP)[:KW, :, :]
            nc.sync.dma_start(out=dst, in_=src)

        if variant == "nomm":
            ysrc = it_all.rearrange("p (w c) -> p w c", c=WP)[:MW, :, PAD : PAD + W]
            ydst = bass.AP(
                tensor=y, offset=b * HOUT * W, ap=[[W, MW], [MW * W, NW], [1, W]]
            )
            nc.scalar.dma_start(out=ydst, in_=ysrc)
            continue

        ot_all = None
        if variant != "nodma":
            ot_all = opool.tile([128, NW * W], out_dt, tag="ot_all", name="ot_all")

        for w in range(NW):
            pss = {}
            for wb in range(W // NBLK):
                ps = pspool.tile([128, NBLK], f32, tag="ps", name="ps")
                pss[wb] = ps
                for idx, dy in enumerate(DYS):
                    j = abs(dy)
                    c0 = w * WP + PAD + NBLK * wb - dy
                    nc.tensor.matmul(
                        ps[:MW, :],
                        lhsT=vbt[:KW, j * MW : j * MW + MW],
                        rhs=it_all[:KW, c0 : c0 + NBLK],
                        start=(idx == 0),
                        stop=(idx == len(DYS) - 1),
                    )
            if variant == "nodma":
                continue
            for wb in range(W // NBLK):
                dstslice = ot_all[:MW, w * W + wb * NBLK : w * W + (wb + 1) * NBLK]
                if (w + wb) % 2 == 0:
                    nc.scalar.copy(dstslice, pss[wb][:MW, :])
                else:
                    nc.vector.tensor_copy(dstslice, pss[wb][:MW, :])

        if variant == "nodma":
            continue
        # one batched output DMA (ACT HWDGE ring)
        ysrc = ot_all.rearrange("p (w c) -> p w c", c=W)[:MW, :, :]
        ydst = bass.AP(
            tensor=y, offset=b * HOUT * W, ap=[[W, MW], [MW * W, NW], [1, W]]
        )
        nc.scalar.dma_start(out=ydst, in_=ysrc)


def _build_program(timing_loop: int = 0, dtype: str | None = None, variant: str = "full"):
    """timing_loop=0: the real kernel (external I/O).
    timing_loop=R>0: same compute on Internal DRAM, looped R times via For_i,
    with a tiny external output — for wall-clock HW timing."""
    from concourse.bacc import Bacc
    from concourse import bass
    import concourse.mybir as mybir
    from concourse.tile import TileContext

    f32 = mybir.dt.float32
    in_dt = getattr(mybir.dt, dtype or DTYPE)
    out_dt = in_dt if variant == "nomm" else getattr(mybir.dt, OUT_DTYPE)

    nc = Bacc("TRN2", target_bir_lowering=False, debug=False)
    kind = "Internal" if timing_loop else None
    x = nc.dram_tensor("x", [B_PER_CORE, HPX, WP], in_dt, kind=kind or "ExternalInput")
    vb = nc.dram_tensor("vb", [128, 6 * MW], in_dt, kind=kind or "ExternalInput")
    y = nc.dram_tensor("y", [B_PER_CORE, HOUT, W], out_dt, kind=kind or "ExternalOutput")
    if timing_loop:
        tout = nc.dram_tensor("tout", [1, 1], out_dt, kind="ExternalOutput")

    with TileContext(nc) as tc:
        with (
            tc.tile_pool(name="bands", bufs=1) as bpool,
            tc.tile_pool(name="inp", bufs=2) as ipool,
            tc.tile_pool(name="outp", bufs=2) as opool,
            tc.tile_pool(name="ps", bufs=8, space="PSUM") as pspool,
        ):
            vbt = bpool.tile([128, 6 * MW], in_dt, name="vbt")
            nc.sync.dma_start(out=vbt[:, :], in_=vb[:, :])
            fixed_it = None
            if variant == "nodma":
                fixed_it = ipool.tile([128, NW * WP], in_dt, name="fixed_it", bufs=1)
                nc.sync.dma_start(out=fixed_it[:, 0:WP], in_=x[0, 0:128, :])
            pools = (ipool, opool, pspool)
            args = (nc, mybir, bass, pools, vbt, x, y, in_dt, out_dt, variant, fixed_it)
            if timing_loop:
                with tc.For_i(0, timing_loop, 1):
                    _emit_body(*args)
                sm = opool.tile([1, 1], out_dt, name="sm")
                nc.sync.dma_start(out=sm[:, :], in_=y[0, 0:1, 0:1])
                nc.sync.dma_start(out=tout[:, :], in_=sm[:, :])
            else:
                _emit_body(*args)
    nc.compile()
    return nc


def _get_program():
    if "nc" not in _CACHE:
        _CACHE["nc"] = _build_program()
        _CACHE["vb"] = _build_vbands()
    return _CACHE["nc"], _CACHE["vb"]


def _run(grid_spikes: np.ndarray, **spmd_kwargs):
    """Run the SPMD kernel on the full (64, 1024, 1024) input.

    Returns (output, BassKernelResults)."""
    from concourse.bass_utils import run_bass_kernel_spmd
    import concourse.mybir as mybir

    nc, vb = _get_program()
    gs = np.ascontiguousarray(grid_spikes, dtype=np.float32)
    assert gs.shape == (B_TOTAL, H, W), gs.shape
    gp = np.pad(gs, ((0, 0), (PAD, 7), (PAD, PAD)), mode="wrap")
    np_in = mybir.dt.np(getattr(mybir.dt, DTYPE))
    gp = gp.astype(np_in)
    vb = vb.astype(np_in)
    in_maps = [
        {"x": gp[c * B_PER_CORE : (c + 1) * B_PER_CORE], "vb": vb}
        for c in range(N_CORES)
    ]
    res = run_bass_kernel_spmd(nc, in_maps, core_ids=list(range(N_CORES)), **spmd_kwargs)
    out = np.concatenate([r["y"][:, :H, :] for r in res.results], axis=0).astype(
        np.float32
    )
    return out, res


def kernel(grid_spikes: np.ndarray) -> np.ndarray:
    out, _ = _run(grid_spikes)
    return out


# revision 21
# speedup vs baseline: 2.2345x; 1.5449x over previous
"""Trainium2 Bass kernel for nn_LocalConnectivity (diamond stencil, B=64, H=W=1024).

out[b,h,w] = sum over offsets (dx,dy), 1 <= |dx|+|dy| <= 5, of
             exp(-(|dx|+|dy|)) * x[b, (h-dx) % H, (w-dy) % W]

Strategy (per core, 8 images each, batch-sharded over 8 NeuronCores):
  Group the 60 taps by horizontal offset dy in [-5, 5]. For each dy the
  vertical tap profile is a banded matrix G_{|dy|} (half-width 5-|dy|).
  The host pre-pads each image circularly (5 left/top/right, 7 bottom) to
  (1036, 1034). Each image is processed as 9 uniform 114-row output
  windows; per window and 512-col block, 11 TensorEngine matmuls
  accumulate into PSUM:
      psum[:114, :] += vband_j[:124, :114].T @ tile[:124, 5+off-dy : +512]
  (bf16 streams at 1 col/cycle). PSUM is evacuated by ScalarE/VectorE
  copies into fp16 tiles and DMA'd back to HBM. Rows 1024..1025 of the
  padded output are garbage and dropped on the host.

  DMA is batched per image (one ~2.4 MB input transfer with overlapping-
  window access patterns, one ~2.1 MB output transfer): small per-window
  DMAs run at <140 GB/s while >=1 MB transfers reach ~340 GB/s. Input
  DMAs ride the SP HWDGE ring, output DMAs the ACT ring.
"""

import math

import numpy as np

B_TOTAL = 64
B_PER_CORE = 8
N_CORES = 8
H = 1024
W = 1024
PAD = 5
HPX = H + PAD + 7  # 1036 (5 top, 7 bottom: 2 extra rows for the 9th window)
WP = W + 2 * PAD  # 1034
MW = 114  # output rows per h-window
NW = 9  # uniform windows per image; NW*MW = 1026 >= H (2 garbage rows)
KW = MW + 10  # 124 input rows per window
HOUT = NW * MW  # 1026 padded output rows
NBLK = 512  # w-block streamed per matmul (PSUM bank = 512 fp32)
DYS = [0, -1, 1, -2, 2, -3, 3, -4, 4, -5, 5]

DTYPE = "float16"  # matmul input dtype: "float16", "bfloat16" or "float32r"
OUT_DTYPE = "float16"  # HBM output dtype: "float16" or "float32"

_CACHE = {}


def _build_vbands() -> np.ndarray:
    """vb[k, j*MW + m] = G_j(dx) at dx = m + 5 - k.

    G_j(dx) = exp(-(|dx|+j)) for |dx| <= 5-j, excluding the (j=0, dx=0) tap.
    lhsT layout: [K=128 partitions, 6*MW free], band j in cols [j*MW, j*MW+MW).
    """
    vb = np.zeros((128, 6 * MW), np.float32)
    for j in range(6):
        for m in range(MW):
            for dx in range(-(5 - j), 5 - j + 1):
                if j == 0 and dx == 0:
                    continue
                k = m + 5 - dx
                vb[k, j * MW + m] = math.exp(-(abs(dx) + j))
    return vb


def _emit_body(nc, mybir, bass, pools, vbt, x, y, in_dt, out_dt, variant="full", fixed_it=None):
    """Emit the per-core compute: all images; batched per-image DMAs.

    variant: "full" | "nodma" (matmuls only, fixed input tile) | "nomm" (DMA only)
    """
    f32 = mybir.dt.float32
    ipool, opool, pspool, tpool = pools

    for b in range(B_PER_CORE):
        if variant == "nodma":
            it_all = fixed_it
        else:
            # one batched input DMA: 9 overlapping 124-row windows
            it_all = ipool.tile([128, NW * WP], in_dt, tag="it_all", name="it_all")
            src = bass.AP(
                tensor=x,
                offset=b * HPX * WP,
                ap=[[WP, KW], [MW * WP, NW], [1, WP]],
            )
            dst = it_all.rearrange("p (w c) -> p w c", c=WP)[:KW, :, :]
            nc.sync.dma_start(out=dst, in_=src)

        if variant == "nomm":
            ysrc = it_all.rearrange("p (w c) -> p w c", c=WP)[:MW, :, PAD : PAD + W]
            ydst = bass.AP(
                tensor=y, offset=b * HOUT * W, ap=[[W, MW], [MW * W, NW], [1, W]]
            )
            nc.scalar.dma_start(out=ydst, in_=ysrc)
            continue

        ot_all = None
        if variant != "nodma":
            ot_all = opool.tile([128, NW * W], out_dt, tag="ot_all", name="ot_all")

        for w in range(NW):
            pss = {}
            for wb in range(W // NBLK):
                ps = pspool.tile([128, NBLK], f32, tag="ps", name="ps")
                pss[wb] = ps
                for idx, dy in enumerate(DYS):
                    j = abs(dy)
                    c0 = w * WP + PAD + NBLK * wb - dy
                    nc.tensor.matmul(
                        ps[:MW, :],
                        lhsT=vbt[:KW, j * MW : j * MW + MW],
                        rhs=it_all[:KW, c0 : c0 + NBLK],
                        start=(idx == 0),
                        stop=(idx == len(DYS) - 1),
                    )
            if variant == "nodma":
                continue
            for wb in range(W // NBLK):
                dstslice = ot_all[:MW, w * W + wb * NBLK : w * W + (wb + 1) * NBLK]
                if (w + wb) % 2 == 0:
                    nc.scalar.copy(dstslice, pss[wb][:MW, :])
                else:
                    nc.vector.tensor_copy(dstslice, pss[wb][:MW, :])

        if variant == "nodma":
            continue
        # one batched output DMA (ACT HWDGE ring)
        ysrc = ot_all.rearrange("p (w c) -> p w c", c=W)[:MW, :, :]
        ydst = bass.AP(
            tensor=y, offset=b * HOUT * W, ap=[[W, MW], [MW * W, NW], [1, W]]
        )
        nc.scalar.dma_start(out=ydst, in_=ysrc)


def _build_program(timing_loop: int = 0, dtype: str | None = None, variant: str = "full"):
    """timing_loop=0: the real kernel (external I/O).
    timing_loop=R>0: same compute on Internal DRAM, looped R times via For_i,
    with a tiny external output — for wall-clock HW timing."""
    from concourse.bacc import Bacc
    from concourse import bass
    import concourse.mybir as mybir
    from concourse.tile import TileContext

    f32 = mybir.dt.float32
    in_dt = getattr(mybir.dt, dtype or DTYPE)
    out_dt = in_dt if variant == "nomm" else getattr(mybir.dt, OUT_DTYPE)

    nc = Bacc("TRN2", target_bir_lowering=False, debug=False)
    kind = "Internal" if timing_loop else None
    x = nc.dram_tensor("x", [B_PER_CORE, HPX, WP], in_dt, kind=kind or "ExternalInput")
    vb = nc.dram_tensor("vb", [128, 6 * MW], in_dt, kind=kind or "ExternalInput")
    y = nc.dram_tensor("y", [B_PER_CORE, HOUT, W], out_dt, kind=kind or "ExternalOutput")
    if timing_loop:
        tout = nc.dram_tensor("tout", [1, 1], out_dt, kind="ExternalOutput")

    with TileContext(nc) as tc:
        with (
            tc.tile_pool(name="bands", bufs=1) as bpool,
            tc.tile_pool(name="inp", bufs=3) as ipool,
            tc.tile_pool(name="outp", bufs=3) as opool,
            tc.tile_pool(name="ps", bufs=6, space="PSUM") as pspool,
            tc.tile_pool(name="tp", bufs=4) as tpool,
        ):
            vbt = bpool.tile([128, 6 * MW], in_dt, name="vbt")
            nc.sync.dma_start(out=vbt[:, :], in_=vb[:, :])
            fixed_it = None
            if variant == "nodma":
                fixed_it = ipool.tile([128, NW * WP], in_dt, name="fixed_it", bufs=1)
                nc.sync.dma_start(out=fixed_it[:, 0:WP], in_=x[0, 0:128, :])
            pools = (ipool, opool, pspool, tpool)
            args = (nc, mybir, bass, pools, vbt, x, y, in_dt, out_dt, variant, fixed_it)
            if timing_loop:
                with tc.For_i(0, timing_loop, 1):
                    _emit_body(*args)
                sm = opool.tile([1, 1], out_dt, name="sm")
                nc.sync.dma_start(out=sm[:, :], in_=y[0, 0:1, 0:1])
                nc.sync.dma_start(out=tout[:, :], in_=sm[:, :])
            else:
                _emit_body(*args)
    nc.compile()
    return nc


def _get_program():
    if "nc" not in _CACHE:
        _CACHE["nc"] = _build_program()
        _CACHE["vb"] = _build_vbands()
    return _CACHE["nc"], _CACHE["vb"]


def _run(grid_spikes: np.ndarray, **spmd_kwargs):
    """Run the SPMD kernel on the full (64, 1024, 1024) input.

    Returns (output, BassKernelResults)."""
    from concourse.bass_utils import run_bass_kernel_spmd
    import concourse.mybir as mybir

    nc, vb = _get_program()
    gs = np.ascontiguousarray(grid_spikes, dtype=np.float32)
    assert gs.shape == (B_TOTAL, H, W), gs.shape
    gp = np.pad(gs, ((0, 0), (PAD, 7), (PAD, PAD)), mode="wrap")
    np_in = mybir.dt.np(getattr(mybir.dt, DTYPE))
    gp = gp.astype(np_in)
    vb = vb.astype(np_in)
    in_maps = [
        {"x": gp[c * B_PER_CORE : (c + 1) * B_PER_CORE], "vb": vb}
        for c in range(N_CORES)
    ]
    res = run_bass_kernel_spmd(nc, in_maps, core_ids=list(range(N_CORES)), **spmd_kwargs)
    out = np.concatenate([r["y"][:, :H, :] for r in res.results], axis=0).astype(
        np.float32
    )
    return out, res


def kernel(grid_spikes: np.ndarray) -> np.ndarray:
    out, _ = _run(grid_spikes)
    return out


# revision 22
# speedup vs baseline: 54239.5273x; 24273.4338x over previous
"""Trainium2 Bass kernel for nn_LocalConnectivity (diamond stencil, B=64, H=W=1024).

out[b,h,w] = sum over offsets (dx,dy), 1 <= |dx|+|dy| <= 5, of
             exp(-(|dx|+|dy|)) * x[b, (h-dx) % H, (w-dy) % W]

Strategy (per core, 8 images each, batch-sharded over 8 NeuronCores):
  Group the 60 taps by horizontal offset dy in [-5, 5]. For each dy the
  vertical tap profile is a banded matrix G_{|dy|} (half-width 5-|dy|).
  The host pre-pads each image circularly (5 left/top/right, 7 bottom) to
  (1036, 1034). Each image is processed as 9 uniform 114-row output
  windows; per window and 512-col block, 11 TensorEngine matmuls
  accumulate into PSUM:
      psum[:114, :] += vband_j[:124, :114].T @ tile[:124, 5+off-dy : +512]
  (bf16 streams at 1 col/cycle). PSUM is evacuated by ScalarE/VectorE
  copies into fp16 tiles and DMA'd back to HBM. Rows 1024..1025 of the
  padded output are garbage and dropped on the host.

  DMA is batched per image (one ~2.4 MB input transfer with overlapping-
  window access patterns, one ~2.1 MB output transfer): small per-window
  DMAs run at <140 GB/s while >=1 MB transfers reach ~340 GB/s. Input
  DMAs ride the SP HWDGE ring, output DMAs the ACT ring.
"""

import math

import numpy as np

B_TOTAL = 64
B_PER_CORE = 8
N_CORES = 8
H = 1024
W = 1024
PAD = 5
HPX = H + PAD + 7  # 1036 (5 top, 7 bottom: 2 extra rows for the 9th window)
WP = W + 2 * PAD  # 1034
MW = 114  # output rows per h-window
NW = 9  # uniform windows per image; NW*MW = 1026 >= H (2 garbage rows)
KW = MW + 10  # 124 input rows per window
HOUT = NW * MW  # 1026 padded output rows
NBLK = 512  # w-block streamed per matmul (PSUM bank = 512 fp32)
# dy=+-5 (single-tap columns) are computed on DVE, not the PE:
DYS = [0, -1, 1, -2, 2, -3, 3, -4, 4]
MB = MW + PAD  # matmul M: 5 zero lhsT columns pad so psum rows align with input partitions
W5 = math.exp(-5.0)

DTYPE = "float16"  # matmul input dtype: "float16", "bfloat16" or "float32r"
OUT_DTYPE = "float16"  # HBM output dtype: "float16" or "float32"

_CACHE = {}


def _build_vbands() -> np.ndarray:
    """vb[k, j*MB + p] = G_j(dx) at dx = p - k, for p in [5, 119).

    G_j(dx) = exp(-(|dx|+j)) for |dx| <= 5-j, excluding the (j=0, dx=0) tap.
    Output partition p maps to window output row m = p - 5 = input-tile
    partition p, so PSUM rows align with input partitions and the dy=+-5
    side contribution can be fused elementwise. Columns p < 5 are zero
    (garbage psum rows 0..4). Only j = 0..4 (dy=+-5 is off-PE).
    """
    vb = np.zeros((128, 5 * MB), np.float32)
    for j in range(5):
        for p in range(PAD, MB):
            for dx in range(-(5 - j), 5 - j + 1):
                if j == 0 and dx == 0:
                    continue
                k = p - dx
                vb[k, j * MB + p] = math.exp(-(abs(dx) + j))
    return vb


def _emit_body(nc, mybir, bass, pools, vbt, x, y, in_dt, out_dt, variant="full", fixed_it=None):
    """Emit the per-core compute: all images; batched per-image DMAs.

    variant: "full" | "nodma" (matmuls only, fixed input tile) | "nomm" (DMA only)
    """
    f32 = mybir.dt.float32
    ipool, opool, pspool, tpool = pools

    for b in range(B_PER_CORE):
        if variant == "nodma":
            it_all = fixed_it
        else:
            # one batched input DMA: 9 overlapping 124-row windows
            it_all = ipool.tile([128, NW * WP], in_dt, tag="it_all", name="it_all")
            src = bass.AP(
                tensor=x,
                offset=b * HPX * WP,
                ap=[[WP, KW], [MW * WP, NW], [1, WP]],
            )
            dst = it_all.rearrange("p (w c) -> p w c", c=WP)[:KW, :, :]
            nc.sync.dma_start(out=dst, in_=src)

        if variant == "nomm":
            ysrc = it_all.rearrange("p (w c) -> p w c", c=WP)[:MW, :, PAD : PAD + W]
            ydst = bass.AP(
                tensor=y, offset=b * HOUT * W, ap=[[W, MW], [MW * W, NW], [1, W]]
            )
            nc.scalar.dma_start(out=ydst, in_=ysrc)
            continue

        ot_all = None
        if variant != "nodma":
            ot_all = opool.tile([128, NW * W], out_dt, tag="ot_all", name="ot_all")

        for w in range(NW):
            pss = {}
            tts = {}
            for wb in range(W // NBLK):
                ps = pspool.tile([128, NBLK], f32, tag="ps", name="ps")
                pss[wb] = ps
                base = w * WP + PAD + NBLK * wb
                for idx, dy in enumerate(DYS):
                    j = abs(dy)
                    nc.tensor.matmul(
                        ps[:MB, :],
                        lhsT=vbt[:KW, j * MB : j * MB + MB],
                        rhs=it_all[:KW, base - dy : base - dy + NBLK],
                        start=(idx == 0),
                        stop=(idx == len(DYS) - 1),
                    )
                if variant == "nodma":
                    continue
                # dy=+-5 single-tap pair sum (DVE, fp16 2x mode)
                t = tpool.tile([128, NBLK], in_dt, tag="t", name="t")
                tts[wb] = t
                nc.vector.tensor_add(
                    t[:MB, :],
                    it_all[:MB, base - 5 : base - 5 + NBLK],
                    it_all[:MB, base + 5 : base + 5 + NBLK],
                )
            if variant == "nodma":
                continue
            for wb in range(W // NBLK):
                # fused evacuation on DVE: ot = (pairsum * e^-5) + psum
                dstslice = ot_all[:MB, w * W + wb * NBLK : w * W + (wb + 1) * NBLK]
                nc.vector.scalar_tensor_tensor(
                    dstslice,
                    tts[wb][:MB, :],
                    W5,
                    pss[wb][:MB, :],
                    mybir.AluOpType.mult,
                    mybir.AluOpType.add,
                )

        if variant == "nodma":
            continue
        # one batched output DMA (ACT HWDGE ring)
        ysrc = ot_all.rearrange("p (w c) -> p w c", c=W)[PAD:MB, :, :]
        ydst = bass.AP(
            tensor=y, offset=b * HOUT * W, ap=[[W, MW], [MW * W, NW], [1, W]]
        )
        nc.scalar.dma_start(out=ydst, in_=ysrc)


def _build_program(timing_loop: int = 0, dtype: str | None = None, variant: str = "full"):
    """timing_loop=0: the real kernel (external I/O).
    timing_loop=R>0: same compute on Internal DRAM, looped R times via For_i,
    with a tiny external output — for wall-clock HW timing."""
    from concourse.bacc import Bacc
    from concourse import bass
    import concourse.mybir as mybir
    from concourse.tile import TileContext

    f32 = mybir.dt.float32
    in_dt = getattr(mybir.dt, dtype or DTYPE)
    out_dt = in_dt if variant == "nomm" else getattr(mybir.dt, OUT_DTYPE)

    nc = Bacc("TRN2", target_bir_lowering=False, debug=False)
    kind = "Internal" if timing_loop else None
    x = nc.dram_tensor("x", [B_PER_CORE, HPX, WP], in_dt, kind=kind or "ExternalInput")
    vb = nc.dram_tensor("vb", [128, 5 * MB], in_dt, kind=kind or "ExternalInput")
    y = nc.dram_tensor("y", [B_PER_CORE, HOUT, W], out_dt, kind=kind or "ExternalOutput")
    if timing_loop:
        tout = nc.dram_tensor("tout", [1, 1], out_dt, kind="ExternalOutput")

    with TileContext(nc) as tc:
        with (
            tc.tile_pool(name="bands", bufs=1) as bpool,
            tc.tile_pool(name="inp", bufs=4) as ipool,
            tc.tile_pool(name="outp", bufs=4) as opool,
            tc.tile_pool(name="ps", bufs=6, space="PSUM") as pspool,
            tc.tile_pool(name="tp", bufs=4) as tpool,
        ):
            vbt = bpool.tile([128, 5 * MB], in_dt, name="vbt")
            nc.sync.dma_start(out=vbt[:, :], in_=vb[:, :])
            fixed_it = None
            if variant == "nodma":
                fixed_it = ipool.tile([128, NW * WP], in_dt, name="fixed_it", bufs=1)
                nc.sync.dma_start(out=fixed_it[:, 0:WP], in_=x[0, 0:128, :])
            pools = (ipool, opool, pspool, tpool)
            args = (nc, mybir, bass, pools, vbt, x, y, in_dt, out_dt, variant, fixed_it)
            if timing_loop:
                with tc.For_i(0, timing_loop, 1):
                    _emit_body(*args)
                sm = opool.tile([1, 1], out_dt, name="sm")
                nc.sync.dma_start(out=sm[:, :], in_=y[0, 0:1, 0:1])
                nc.sync.dma_start(out=tout[:, :], in_=sm[:, :])
            else:
                _emit_body(*args)
    nc.compile()
    return nc


def _get_program():
    if "nc" not in _CACHE:
        _CACHE["nc"] = _build_program()
        _CACHE["vb"] = _build_vbands()
    return _CACHE["nc"], _CACHE["vb"]


def _run(grid_spikes: np.ndarray, **spmd_kwargs):
    """Run the SPMD kernel on the full (64, 1024, 1024) input.

    Returns (output, BassKernelResults)."""
    from concourse.bass_utils import run_bass_kernel_spmd
    import concourse.mybir as mybir

    nc, vb = _get_program()
    gs = np.ascontiguousarray(grid_spikes, dtype=np.float32)
    assert gs.shape == (B_TOTAL, H, W), gs.shape
    gp = np.pad(gs, ((0, 0), (PAD, 7), (PAD, PAD)), mode="wrap")
    np_in = mybir.dt.np(getattr(mybir.dt, DTYPE))
    gp = gp.astype(np_in)
    vb = vb.astype(np_in)
    in_maps = [
        {"x": gp[c * B_PER_CORE : (c + 1) * B_PER_CORE], "vb": vb}
        for c in range(N_CORES)
    ]
    res = run_bass_kernel_spmd(nc, in_maps, core_ids=list(range(N_CORES)), **spmd_kwargs)
    out = np.concatenate([r["y"][:, :H, :] for r in res.results], axis=0).astype(
        np.float32
    )
    return out, res


def kernel(grid_spikes: np.ndarray) -> np.ndarray:
    out, _ = _run(grid_spikes)
    return out
